# revision 3
# baseline (speedup 1.0000x reference)
"""Trainium2 Bass kernel for nn_AtteMatchLay (multi-perspective cosine matching).

Math (per flattened row n, perspective p):
    dot[n,p] = sum_d r[n,d]*m[n,d]*w2[p,d]
    n1s[n,p] = sum_d r[n,d]^2 * w2[p,d]        (w2 = weight**2)
    n2s[n,p] = sum_d m[n,d]^2 * w2[p,d]
    cos[n,p] = dot / (sqrt(n1s)*sqrt(n2s))

Strategy: data-parallel over the flattened N=16*512=8192 rows across 8 cores
(1024 rows each). Host hands each core its shard TRANSPOSED and d-block-packed
([128, 6*1024] bf16, partition-major) so the contraction dim D sits on SBUF
partitions and every DMA is 128 fat contiguous descriptors.

Key scheduling facts this version exploits (from trace + cost-model analysis):
  * DVE's 2-port perf mode shares an exclusive-lock SBUF port pair with
    GpSimd - concurrent GpSimd tensor ops stall DVE 4x. So products run on
    DVE+ACT only; GpSimd does nothing.
  * Both SP and ACT queues can trigger HWDGE DMAs - triggers are issued from
    both in parallel (r-chunks on SP, m-chunks on ACT) so ~600ns/trigger
    generation never serializes against the stream.
  * The abs_reciprocal_sqrt_and_small ACT table also contains `square`; a
    dummy ARSQRT issued before any Square makes the one table load happen
    during the DMA-wait window instead of a 1.3us reload on the tail.
  * Per-block DMA chunks (b0..b4 full, b5 in column quarters) keep products
    and matmuls streaming; the last quarter only gates [128,256] products,
    3 matmuls and a short epilogue slice.
"""

import sys

if "/opt/trn_rl_repo" not in sys.path:
    sys.path.insert(0, "/opt/trn_rl_repo")

import numpy as np

# ---- problem constants (hardcoded per contract) ----
BSZ, SL, D, MP = 16, 512, 768, 20
N = BSZ * SL           # 8192 flattened rows
NCORES = 8
NSH = N // NCORES      # 1024 rows per core
P = 128                # SBUF partitions
NB = D // P            # 6 d-blocks
G = 2                  # PSUM column groups (512 each)
GW = NSH // G          # 512
Q = 4                  # tail quarters of block 5
QW = NSH // Q          # 256

_CACHE = {}


def _build():
    import concourse.tile as tile
    from concourse import bacc, mybir

    f32 = mybir.dt.float32
    bf16 = mybir.dt.bfloat16
    nc = bacc.Bacc(None, target_bir_lowering=False)

    rD = nc.dram_tensor("rD", [P, NB * NSH], bf16, kind="ExternalInput")
    mD = nc.dram_tensor("mD", [P, NB * NSH], bf16, kind="ExternalInput")
    w2D = nc.dram_tensor("w2D", [P, NB * MP], bf16, kind="ExternalInput")
    out = nc.dram_tensor("out", [MP, NSH], bf16, kind="ExternalOutput")

    SQ = mybir.ActivationFunctionType.Square
    ARSQRT = mybir.ActivationFunctionType.Abs_reciprocal_sqrt
    MUL = mybir.AluOpType.mult

    with tile.TileContext(nc) as tc:
        with (
            tc.tile_pool(name="const", bufs=1) as const,
            tc.tile_pool(name="inp", bufs=1) as inp,
            tc.tile_pool(name="prod", bufs=2) as prod,
            tc.tile_pool(name="epi", bufs=1) as epi,
            tc.tile_pool(name="psum", bufs=1, space="PSUM") as psum,
        ):
            w2_sb = const.tile([P, NB, MP], bf16, tag="w2")
            r_sb = inp.tile([P, NB, NSH], bf16, tag="r")
            m_sb = inp.tile([P, NB, NSH], bf16, tag="m")
            # activation-bias tiles initialized on-chip so no const tensor
            # needs a DRAM preamble load
            bias_b = const.tile([P, 1], bf16, tag="bias_b")
            bias_f = const.tile([MP, 1], f32, tag="bias_f")
            dum_in = const.tile([1, 1], f32, tag="dum_in")
            dum_out = const.tile([1, 1], f32, tag="dum_out")
            nc.gpsimd.memset(bias_b[:], 0.0)
            nc.gpsimd.memset(bias_f[:], 0.0)
            nc.gpsimd.memset(dum_in[:], 1.0)

            # Force the abs_reciprocal_sqrt_and_small table (which also
            # serves Square) to load once, now, hidden under the DMA wait.
            nc.scalar.activation(dum_out[:], dum_in[:], ARSQRT, bias=0.0)

            # DMA triggers, r-chunks from the SP queue and m-chunks from the
            # ACT queue so trigger generation runs two-wide.
            nc.sync.dma_start(out=r_sb[:, 0, :], in_=rD[:, 0:NSH])
            nc.scalar.dma_start(out=w2_sb[:], in_=w2D[:, :].rearrange("p (b q) -> p b q", b=NB))
            nc.scalar.dma_start(out=m_sb[:, 0, :], in_=mD[:, 0:NSH])
            for b in range(1, NB - 1):
                sl = slice(b * NSH, (b + 1) * NSH)
                nc.sync.dma_start(out=r_sb[:, b, :], in_=rD[:, sl])
                nc.scalar.dma_start(out=m_sb[:, b, :], in_=mD[:, sl])
            for q in range(Q):
                sl = slice((NB - 1) * NSH + q * QW, (NB - 1) * NSH + (q + 1) * QW)
                qc = slice(q * QW, (q + 1) * QW)
                nc.sync.dma_start(out=r_sb[:, NB - 1, qc], in_=rD[:, sl])
                nc.scalar.dma_start(out=m_sb[:, NB - 1, qc], in_=mD[:, sl])

            # PSUM accumulators: 2 column groups x {dot, n1, n2}
            dot_ps, n1_ps, n2_ps = [], [], []
            for g in range(G):
                dps = psum.tile([MP, GW], f32, tag=f"dot{g}")
                n1p = psum.tile([MP, GW], f32, tag=f"n1{g}")
                n2p = psum.tile([MP, GW], f32, tag=f"n2{g}")
                dot_ps.append(dps)
                n1_ps.append(n1p)
                n2_ps.append(n2p)

            # Blocks 0..4: full-width products (DVE: rm+mm, ACT: rr), then
            # 6 accumulating matmuls per block.
            for b in range(NB - 1):
                rsl = r_sb[:, b, :]
                msl = m_sb[:, b, :]
                rm = prod.tile([P, NSH], bf16, tag="rm")
                rr = prod.tile([P, NSH], bf16, tag="rr")
                mm = prod.tile([P, NSH], bf16, tag="mm")
                nc.vector.tensor_tensor(rm[:], rsl, msl, MUL)
                nc.scalar.activation(rr[:], rsl, SQ, bias=bias_b[:])
                nc.vector.tensor_tensor(mm[:], msl, msl, MUL)

                w2b = w2_sb[:, b, :]
                st = b == 0
                for g in range(G):
                    gsl = slice(g * GW, (g + 1) * GW)
                    nc.tensor.matmul(dot_ps[g][:], w2b, rm[:, gsl], start=st, stop=False)
                    nc.tensor.matmul(n1_ps[g][:], w2b, rr[:, gsl], start=st, stop=False)
                    nc.tensor.matmul(n2_ps[g][:], w2b, mm[:, gsl], start=st, stop=False)

            # Block 5 in column quarters: products sliced per quarter so each
            # landing quarter immediately feeds its 3 finishing matmuls.
            rm5 = prod.tile([P, NSH], bf16, tag="rm5")
            rr5 = prod.tile([P, NSH], bf16, tag="rr5")
            mm5 = prod.tile([P, NSH], bf16, tag="mm5")
            w2b5 = w2_sb[:, NB - 1, :]
            b5 = NB - 1
            for q in range(Q):
                qc = slice(q * QW, (q + 1) * QW)
                g = q // (Q // G)
                gq = slice((q * QW) % GW, (q * QW) % GW + QW)
                nc.vector.tensor_tensor(rm5[:, qc], r_sb[:, b5, qc], m_sb[:, b5, qc], MUL)
                nc.scalar.activation(rr5[:, qc], r_sb[:, b5, qc], SQ, bias=bias_b[:])
                nc.vector.tensor_tensor(mm5[:, qc], m_sb[:, b5, qc], m_sb[:, b5, qc], MUL)
                nc.tensor.matmul(dot_ps[g][:, gq], w2b5, rm5[:, qc],
                                 start=False, stop=q % 2 == 1, skip_group_check=True)
                nc.tensor.matmul(n1_ps[g][:, gq], w2b5, rr5[:, qc],
                                 start=False, stop=q % 2 == 1, skip_group_check=True)
                nc.tensor.matmul(n2_ps[g][:, gq], w2b5, mm5[:, qc],
                                 start=False, stop=q % 2 == 1, skip_group_check=True)

            # Epilogue per group-half: u1=arsqrt(n1), u2=arsqrt(n2) on ACT
            # (PSUM-direct reads), t=u1*u2 and cos=dot*t on DVE; bf16 out.
            cos = epi.tile([MP, NSH], bf16, tag="cos")
            for g in range(G):
                gsl = slice(g * GW, (g + 1) * GW)
                u1 = epi.tile([MP, GW], f32, tag=f"u1{g}")
                u2 = epi.tile([MP, GW], f32, tag=f"u2{g}")
                t = epi.tile([MP, GW], f32, tag=f"t{g}")
                nc.scalar.activation(u1[:], n1_ps[g][:], ARSQRT, bias=bias_f[:])
                nc.scalar.activation(u2[:], n2_ps[g][:], ARSQRT, bias=bias_f[:])
                nc.vector.tensor_tensor(t[:], u1[:], u2[:], MUL)
                nc.vector.tensor_tensor(cos[:, gsl], dot_ps[g][:], t[:], MUL)
                nc.sync.dma_start(out=out[:, gsl], in_=cos[:, gsl])

    nc.finalize()
    return nc


def get_nc():
    if "nc" not in _CACHE:
        _CACHE["nc"] = _build()
    return _CACHE["nc"]


def _pack(x2d):
    # [1024 rows, 768] f32 -> [128, 6*1024] bf16 with [p, b*1024+n] = x[n, b*128+p]
    import ml_dtypes

    xt = x2d.T.reshape(NB, P, NSH).transpose(1, 0, 2).reshape(P, NB * NSH)
    return np.ascontiguousarray(xt.astype(ml_dtypes.bfloat16))


def make_in_maps(repres, max_att, weight):
    import ml_dtypes

    r = np.ascontiguousarray(repres, dtype=np.float32).reshape(N, D)
    m = np.ascontiguousarray(max_att, dtype=np.float32).reshape(N, D)
    w2t = (weight.astype(np.float32) ** 2).T  # [D, MP]
    w2d = np.ascontiguousarray(
        w2t.reshape(NB, P, MP).transpose(1, 0, 2).reshape(P, NB * MP)
        .astype(ml_dtypes.bfloat16)
    )
    in_maps = []
    for c in range(NCORES):
        rows = slice(c * NSH, (c + 1) * NSH)
        in_maps.append(
            {"rD": _pack(r[rows]), "mD": _pack(m[rows]), "w2D": w2d}
        )
    return in_maps


def gather(results):
    # results: list of dicts with "out" [MP, NSH] bf16 per core -> [BSZ, SL, MP] f32
    cols = np.concatenate(
        [results[c]["out"].astype(np.float32) for c in range(NCORES)], axis=1
    )
    return np.ascontiguousarray(cols.T).reshape(BSZ, SL, MP)


def kernel(repres, max_att, weight, **kw):
    from concourse.bass_utils import run_bass_kernel_spmd

    nc = get_nc()
    in_maps = make_in_maps(repres, max_att, weight)
    res = run_bass_kernel_spmd(nc, in_maps, list(range(NCORES)))
    return gather(res.results)


# revision 4
# speedup vs baseline: 1.1942x; 1.1942x over previous
"""Trainium2 Bass kernel for nn_AtteMatchLay (multi-perspective cosine matching).

Math (per flattened row n, perspective p):
    dot[n,p] = sum_d r[n,d]*m[n,d]*w2[p,d]
    n1s[n,p] = sum_d r[n,d]^2 * w2[p,d]        (w2 = weight**2)
    n2s[n,p] = sum_d m[n,d]^2 * w2[p,d]
    cos[n,p] = dot / (sqrt(n1s)*sqrt(n2s))

Strategy: data-parallel over the flattened N=16*512=8192 rows across 8 cores
(1024 rows each), contraction dim D on SBUF partitions (6 blocks of 128).

Scheduling facts this version is built around (trace + cost-model driven):
  * DVE's 2-port perf mode shares an exclusive-lock SBUF port pair with
    GpSimd; concurrent GpSimd tensor ops stall DVE ~4x. Products run on
    DVE (rm, mm) + ACT (rr squares) only.
  * The ACT sequencer's exec queue depth is 0, so DMA triggers issued from
    the ACT queue serialize against ACT engine ops. All triggers go on the
    SP queue only.
  * One DMA trigger costs ~0.6us of issuing-queue time. r and m are packed
    interleaved per d-block into ONE DRAM tensor so a single trigger lands
    a matched (r,m) block pair: 10 input triggers total, strict pair order,
    4KB descriptors.
  * The abs_reciprocal_sqrt_and_small ACT table also serves `square`; a
    dummy ARSQRT before any Square makes the single table load happen
    during the DMA-wait window instead of a 1.3us reload on the tail.
  * Block 5 streams in column quarters; matmul finishers, the epilogue
    (u1,u2 = arsqrt(n1,n2) on ACT, t=u1*u2, cos=dot*t on DVE) and the two
    output DMAs all run at quarter/half granularity so the last-arriving
    64KB only gates a short chain.
"""

import sys

if "/opt/trn_rl_repo" not in sys.path:
    sys.path.insert(0, "/opt/trn_rl_repo")

import numpy as np

# ---- problem constants (hardcoded per contract) ----
BSZ, SL, D, MP = 16, 512, 768, 20
N = BSZ * SL           # 8192 flattened rows
NCORES = 8
NSH = N // NCORES      # 1024 rows per core
P = 128                # SBUF partitions
NB = D // P            # 6 d-blocks
NBF = NB - 1           # 5 full blocks (b0..b4)
G = 2                  # PSUM column groups (512 each)
GW = NSH // G          # 512
Q = 4                  # tail quarters of block 5
QW = NSH // Q          # 256

_CACHE = {}


def _build():
    import concourse.tile as tile
    from concourse import bacc, mybir

    f32 = mybir.dt.float32
    bf16 = mybir.dt.bfloat16
    nc = bacc.Bacc(None, target_bir_lowering=False)

    # xD packs r and m interleaved: 5 full blocks of [r(1024)|m(1024)] then
    # 4 quarter chunks of block 5 as [r(256)|m(256)].
    xD = nc.dram_tensor("xD", [P, 2 * NB * NSH], bf16, kind="ExternalInput")
    w2D = nc.dram_tensor("w2D", [P, NB * MP], bf16, kind="ExternalInput")
    out = nc.dram_tensor("out", [MP, NSH], bf16, kind="ExternalOutput")

    SQ = mybir.ActivationFunctionType.Square
    ARSQRT = mybir.ActivationFunctionType.Abs_reciprocal_sqrt
    MUL = mybir.AluOpType.mult

    with tile.TileContext(nc) as tc:
        with (
            tc.tile_pool(name="const", bufs=1) as const,
            tc.tile_pool(name="inp", bufs=1) as inp,
            tc.tile_pool(name="prod", bufs=2) as prod,
            tc.tile_pool(name="epi", bufs=1) as epi,
            tc.tile_pool(name="psum", bufs=1, space="PSUM") as psum,
        ):
            w2_sb = const.tile([P, NB, MP], bf16, tag="w2")
            x_sb = inp.tile([P, NBF, 2, NSH], bf16, tag="x")      # b0..b4
            x5_sb = inp.tile([P, Q, 2, QW], bf16, tag="x5")       # b5 quarters
            # activation-bias tiles initialized on-chip so no const tensor
            # needs a DRAM preamble load
            bias_b = const.tile([P, 1], bf16, tag="bias_b")
            bias_f = const.tile([MP, 1], f32, tag="bias_f")
            dum = const.tile([MP, 1], f32, tag="dum")
            nc.gpsimd.memset(bias_b[:], 0.0)
            nc.gpsimd.memset(bias_f[:], 0.0)
            nc.gpsimd.memset(dum[:], 1.0)

            # Force the abs_reciprocal_sqrt_and_small table (which also
            # serves Square) to load once, now, hidden under the DMA wait.
            nc.scalar.activation(dum[:], dum[:], ARSQRT, bias=bias_f[:])

            # All DMA triggers on the SP queue, in stream order.
            nc.sync.dma_start(
                out=w2_sb[:], in_=w2D[:, :].rearrange("p (b q) -> p b q", b=NB)
            )
            for b in range(NBF):
                nc.sync.dma_start(
                    out=x_sb[:, b, :, :],
                    in_=xD[:, 2 * b * NSH : 2 * (b + 1) * NSH].rearrange(
                        "p (t n) -> p t n", t=2
                    ),
                )
            q5base = 2 * NBF * NSH
            for q in range(Q):
                nc.sync.dma_start(
                    out=x5_sb[:, q, :, :],
                    in_=xD[:, q5base + 2 * q * QW : q5base + 2 * (q + 1) * QW]
                    .rearrange("p (t n) -> p t n", t=2),
                )

            # PSUM accumulators: 2 column groups x {dot, n1, n2}
            dot_ps, n1_ps, n2_ps = [], [], []
            for g in range(G):
                dps = psum.tile([MP, GW], f32, tag=f"dot{g}")
                n1p = psum.tile([MP, GW], f32, tag=f"n1{g}")
                n2p = psum.tile([MP, GW], f32, tag=f"n2{g}")
                dot_ps.append(dps)
                n1_ps.append(n1p)
                n2_ps.append(n2p)

            # Blocks 0..4: full-width products (DVE: rm+mm, ACT: rr), then
            # 6 accumulating matmuls per block.
            for b in range(NBF):
                rsl = x_sb[:, b, 0, :]
                msl = x_sb[:, b, 1, :]
                rm = prod.tile([P, NSH], bf16, tag="rm")
                rr = prod.tile([P, NSH], bf16, tag="rr")
                mm = prod.tile([P, NSH], bf16, tag="mm")
                nc.vector.tensor_tensor(rm[:], rsl, msl, MUL)
                nc.scalar.activation(rr[:], rsl, SQ, bias=bias_b[:])
                nc.vector.tensor_tensor(mm[:], msl, msl, MUL)

                w2b = w2_sb[:, b, :]
                st = b == 0
                for g in range(G):
                    gsl = slice(g * GW, (g + 1) * GW)
                    nc.tensor.matmul(dot_ps[g][:], w2b, rm[:, gsl], start=st, stop=False)
                    nc.tensor.matmul(n1_ps[g][:], w2b, rr[:, gsl], start=st, stop=False)
                    nc.tensor.matmul(n2_ps[g][:], w2b, mm[:, gsl], start=st, stop=False)

            # Block 5 quarters: products, finishing matmuls, then that
            # quarter's epilogue slice as soon as its group data is final.
            rm5 = prod.tile([P, NSH], bf16, tag="rm5")
            rr5 = prod.tile([P, NSH], bf16, tag="rr5")
            mm5 = prod.tile([P, NSH], bf16, tag="mm5")
            cos = epi.tile([MP, NSH], bf16, tag="cos")
            u1 = epi.tile([MP, NSH], f32, tag="u1")
            u2 = epi.tile([MP, NSH], f32, tag="u2")
            t = epi.tile([MP, NSH], f32, tag="t")
            w2b5 = w2_sb[:, NB - 1, :]
            for q in range(Q):
                qc = slice(q * QW, (q + 1) * QW)
                g = q // (Q // G)
                gq = slice((q * QW) % GW, (q * QW) % GW + QW)
                rq = x5_sb[:, q, 0, :]
                mq = x5_sb[:, q, 1, :]
                nc.vector.tensor_tensor(rm5[:, qc], rq, mq, MUL)
                nc.scalar.activation(rr5[:, qc], rq, SQ, bias=bias_b[:])
                nc.vector.tensor_tensor(mm5[:, qc], mq, mq, MUL)
                nc.tensor.matmul(dot_ps[g][:, gq], w2b5, rm5[:, qc],
                                 start=False, stop=q % 2 == 1, skip_group_check=True)
                nc.tensor.matmul(n1_ps[g][:, gq], w2b5, rr5[:, qc],
                                 start=False, stop=q % 2 == 1, skip_group_check=True)
                nc.tensor.matmul(n2_ps[g][:, gq], w2b5, mm5[:, qc],
                                 start=False, stop=q % 2 == 1, skip_group_check=True)
                # epilogue slice for this quarter (PSUM-direct ACT reads)
                nc.scalar.activation(u1[:, qc], n1_ps[g][:, gq], ARSQRT, bias=bias_f[:])
                nc.scalar.activation(u2[:, qc], n2_ps[g][:, gq], ARSQRT, bias=bias_f[:])
                nc.vector.tensor_tensor(t[:, qc], u1[:, qc], u2[:, qc], MUL)
                nc.vector.tensor_tensor(cos[:, qc], dot_ps[g][:, gq], t[:, qc], MUL)
                if q % 2 == 1:
                    h = q // 2
                    hsl = slice(h * GW, (h + 1) * GW)
                    nc.sync.dma_start(out=out[:, hsl], in_=cos[:, hsl])

    nc.finalize()
    return nc


def get_nc():
    if "nc" not in _CACHE:
        _CACHE["nc"] = _build()
    return _CACHE["nc"]


def _pack_pair(r2d, m2d):
    # [1024 rows, 768] f32 x2 -> [128, 12288] bf16: 5 blocks of [r|m] at 1024
    # cols each, then 4 quarter chunks of block 5 as [r|m] at 256 cols each.
    import ml_dtypes

    rt = r2d.T.reshape(NB, P, NSH)  # [b, p, n]
    mt = m2d.T.reshape(NB, P, NSH)
    parts = []
    for b in range(NBF):
        parts.append(rt[b])  # [P, NSH]
        parts.append(mt[b])
    for q in range(Q):
        qc = slice(q * QW, (q + 1) * QW)
        parts.append(rt[NB - 1][:, qc])
        parts.append(mt[NB - 1][:, qc])
    x = np.concatenate(parts, axis=1)  # [P, 2*NB*NSH]
    return np.ascontiguousarray(x.astype(ml_dtypes.bfloat16))


def make_in_maps(repres, max_att, weight):
    import ml_dtypes

    r = np.ascontiguousarray(repres, dtype=np.float32).reshape(N, D)
    m = np.ascontiguousarray(max_att, dtype=np.float32).reshape(N, D)
    w2t = (weight.astype(np.float32) ** 2).T  # [D, MP]
    w2d = np.ascontiguousarray(
        w2t.reshape(NB, P, MP).transpose(1, 0, 2).reshape(P, NB * MP)
        .astype(ml_dtypes.bfloat16)
    )
    in_maps = []
    for c in range(NCORES):
        rows = slice(c * NSH, (c + 1) * NSH)
        in_maps.append({"xD": _pack_pair(r[rows], m[rows]), "w2D": w2d})
    return in_maps


def gather(results):
    # results: list of dicts with "out" [MP, NSH] bf16 per core -> [BSZ, SL, MP] f32
    cols = np.concatenate(
        [results[c]["out"].astype(np.float32) for c in range(NCORES)], axis=1
    )
    return np.ascontiguousarray(cols.T).reshape(BSZ, SL, MP)


def kernel(repres, max_att, weight, **kw):
    from concourse.bass_utils import run_bass_kernel_spmd

    nc = get_nc()
    in_maps = make_in_maps(repres, max_att, weight)
    res = run_bass_kernel_spmd(nc, in_maps, list(range(NCORES)))
    return gather(res.results)
